# revision 25
# baseline (speedup 1.0000x reference)
"""Trainium2 Bass kernel for nn_ATSearcher: repetition penalty + log_softmax +
beam-score add + per-batch top-16, replicating the reference's (neuron jax)
top_k window semantics exactly.

Self-contained: hardcodes shapes (scores [256,128000], beams [256],
input_ids [256,128], num_beams=8) and the 8-core row sharding (32 rows/core,
plus a 1-row halo for the cross-batch window spill).
"""
import sys
sys.path.insert(0, "/opt/trn_rl_repo")

import numpy as np
import concourse.bass as bass
import concourse.bacc as bacc
import concourse.mybir as mybir
import concourse.tile as tile
from concourse.bass_utils import run_bass_kernel_spmd

F32 = mybir.dt.float32
I32 = mybir.dt.int32
U32 = mybir.dt.uint32

P = 128
TOK = 8            # beams per batch group
SUBS = 16          # partitions per beam in the group layout
COLS = 8000        # free cols per group partition (V / SUBS)
W = 16             # segment width
M = COLS // W      # 500 segments per partition per group
V = 128000
NROW = 33          # 32 own rows + 1 halo row
NEG = -1.0e30
# neuron top_k window-bug constants (row len 1024000 padded to 1024002):
S_DIRECT = 1007748
R_TAIL = 1023938
SHIFT = 16190
EX = 160           # extra candidates per group (head/zone-edge/tail/spill)
NFIN = 256 + EX    # final merge width per group

REP = 1.2

_prog_cache = {}


# --------------------------------------------------------------------------
# host-side index-only prep (aux tensors per core)
# --------------------------------------------------------------------------
def _host_prep(core, ids33):
    aux = {}
    aux["ids_off"] = (np.arange(NROW)[None, :] * V + ids33.T).astype(np.int32)

    mm = np.zeros((P, 4 * M), np.float32)
    extras = [[] for _ in range(4)]
    for g in range(4):
        B = 4 * core + g
        b2 = 2 * B
        # head: beam0 v < 2B dropped; mask whole partial segments, re-add edge
        hseg = -(-b2 // W)
        mm[0, g * M: g * M + hseg] = NEG
        for v in range(b2, hseg * W):
            extras[g].append(((8 * g) * V + v, v - b2, 8 * g))
        if B < 31:
            z0 = b2 + 111748          # beam7-local v of zone start
            s0 = z0 // W
            # mask beam7 segments from zone start to end of beam7
            for s in range(s0, SUBS * M):
                sub = (s * W) // COLS
                mm[112 + sub, g * M + (s - sub * M)] = NEG
            # zone-start partial segment elements (included, direct region)
            for v in range(s0 * W, z0):
                extras[g].append(((8 * g + 7) * V + v, 7 * V + v - b2, 8 * g + 7))
            # beam7 tail (included via tail window, remapped by -SHIFT)
            for v in range(b2 + 127938, V):
                extras[g].append(((8 * g + 7) * V + v, 7 * V + v - b2 - SHIFT, 8 * g + 7))
            # spill into next batch's beam0: v in [0, 2B+2)
            nrow = 8 * g + 8 if g < 3 else 32
            for v in range(b2 + 2):
                extras[g].append((nrow * V + v, TOK * V + v - b2 - SHIFT, nrow))
        # B == 31: no zone/tail/spill; everything (except head) direct
    aux["mapmask"] = mm

    eo = np.zeros((4, EX), np.int32)
    ew = np.zeros((4, EX), np.int32)
    er = np.full((4, EX), 32, np.int32)
    em = np.full((4, EX), NEG, np.float32)
    for g in range(4):
        assert len(extras[g]) <= EX, len(extras[g])
        for k, (off, word, row) in enumerate(extras[g]):
            eo[g, k], ew[g, k], er[g, k] = off, word, row
            em[g, k] = 0.0
    # pack extras partition-major: partition q=32g+j holds extras [5j..5j+5)
    aux["extra_off"] = eo.reshape(4, 32, 5).reshape(P, 5)
    aux["extra_words"] = ew.reshape(4, 32, 5).reshape(P, 5)
    aux["extra_row"] = er.reshape(4, 32, 5).reshape(P, 5)
    aux["extra_mask"] = em.reshape(4, 32, 5).reshape(P, 5)

    # small constants
    seg16 = np.zeros((P, 8), np.float32)
    for p in range(P):
        seg16[p, p // 16] = 1.0
    aux["seg16"] = seg16
    aux["t16"] = seg16.T.copy()

    q = np.arange(64)
    c64 = np.stack([
        (q // 16) * 2048,              # gbase (into SEGSCR flat)
        8 * (q // 16),                 # g8
        2 * (4 * core + q // 16),      # 2B per selected-segment slot
        (q // 16) * NFIN,              # qbase (into WRDSCR flat)
    ], axis=1).astype(np.int32)
    aux["c64"] = c64
    aux["k16"] = np.broadcast_to(np.arange(W, dtype=np.int32), (64, W)).copy()
    return aux


# --------------------------------------------------------------------------
# device program (SPMD, one build shared by all cores)
# --------------------------------------------------------------------------
def _build_program():
    nc = bacc.Bacc("TRN2", target_bir_lowering=False, debug=False)

    sc = nc.dram_tensor("scores33", [NROW, V], F32, kind="ExternalInput")
    ids_off = nc.dram_tensor("ids_off", [P, NROW], I32, kind="ExternalInput")
    beams = nc.dram_tensor("beams33", [NROW, 1], F32, kind="ExternalInput")
    mapmask = nc.dram_tensor("mapmask", [P, 4 * M], F32, kind="ExternalInput")
    extra_off = nc.dram_tensor("extra_off", [P, 5], I32, kind="ExternalInput")
    extra_row = nc.dram_tensor("extra_row", [P, 5], I32, kind="ExternalInput")
    extra_words = nc.dram_tensor("extra_words", [P, 5], I32, kind="ExternalInput")
    extra_mask = nc.dram_tensor("extra_mask", [P, 5], F32, kind="ExternalInput")
    seg16 = nc.dram_tensor("seg16", [P, 8], F32, kind="ExternalInput")
    t16 = nc.dram_tensor("t16", [8, P], F32, kind="ExternalInput")
    c64 = nc.dram_tensor("c64", [64, 4], I32, kind="ExternalInput")
    k16 = nc.dram_tensor("k16", [64, W], I32, kind="ExternalInput")
    ones128 = nc.dram_tensor("ones128", [P, 1], F32, kind="ExternalInput")

    out_s = nc.dram_tensor("out_scores", [4, 16], F32, kind="ExternalOutput")
    out_w = nc.dram_tensor("out_words", [4, 16], I32, kind="ExternalOutput")

    offscr = nc.dram_tensor("OFFSCR", [NROW, 1], F32)
    segscr = nc.dram_tensor("SEGSCR", [4, 2048], U32)
    wrdscr = nc.dram_tensor("WRDSCR", [4, NFIN], I32)

    sc_flat = sc[:].rearrange("a b -> (a b)").unsqueeze(1)
    segscr_flat = segscr[:].rearrange("a b -> (a b)").unsqueeze(1)
    wrdscr_flat = wrdscr[:].rearrange("a b -> (a b)").unsqueeze(1)
    offscr_flat = offscr[:]  # [33, 1] already rows

    NCH = 4                     # col chunks per group
    CW = COLS // NCH            # 2000

    with tile.TileContext(nc) as tc:
        with (
            tc.tile_pool(name="io", bufs=1) as io,
            tc.tile_pool(name="big", bufs=3) as big,
            tc.tile_pool(name="scr", bufs=2) as scr,
            tc.tile_pool(name="grp", bufs=2) as grp,
            tc.tile_pool(name="sml", bufs=1) as sml,
            tc.tile_pool(name="ps", bufs=2, space="PSUM") as ps,
        ):
            # ---------- phase 1: repetition penalty (in-place in DRAM) ----
            idst = io.tile([P, NROW], I32)
            nc.sync.dma_start(out=idst[:], in_=ids_off[:])
            gat = io.tile([P, NROW], F32)
            for r in range(NROW):
                nc.gpsimd.indirect_dma_start(
                    out=gat[:, r:r + 1], out_offset=None, in_=sc_flat,
                    in_offset=bass.IndirectOffsetOnAxis(ap=idst[:, r:r + 1], axis=0),
                )
            ga = io.tile([P, NROW], F32)
            gb = io.tile([P, NROW], F32)
            pen = io.tile([P, NROW], F32)
            nc.vector.tensor_scalar_mul(ga[:], gat[:], REP)
            nc.vector.tensor_scalar_mul(gb[:], gat[:], 1.0 / REP)
            nc.vector.tensor_tensor(out=pen[:], in0=ga[:], in1=gb[:],
                                    op=mybir.AluOpType.min)
            for r in range(NROW):
                nc.gpsimd.indirect_dma_start(
                    out=sc_flat,
                    out_offset=bass.IndirectOffsetOnAxis(ap=idst[:, r:r + 1], axis=0),
                    in_=pen[:, r:r + 1], in_offset=None,
                )

            # preload small constants
            mm_t = io.tile([P, 4 * M], F32)
            nc.sync.dma_start(out=mm_t[:], in_=mapmask[:])
            seg16_t = io.tile([P, 8], F32)
            nc.sync.dma_start(out=seg16_t[:], in_=seg16[:])
            t16_t = io.tile([8, P], F32)
            nc.sync.dma_start(out=t16_t[:], in_=t16[:])
            c64_t = io.tile([64, 4], I32)
            nc.sync.dma_start(out=c64_t[:], in_=c64[:])
            k16_t = io.tile([64, W], I32)
            nc.sync.dma_start(out=k16_t[:], in_=k16[:])
            exoff_t = io.tile([P, 5], I32)
            nc.sync.dma_start(out=exoff_t[:], in_=extra_off[:])
            exrow_t = io.tile([P, 5], I32)
            nc.sync.dma_start(out=exrow_t[:], in_=extra_row[:])
            exwrd_t = io.tile([P, 5], I32)
            nc.sync.dma_start(out=exwrd_t[:], in_=extra_words[:])
            exmsk_t = io.tile([P, 5], F32)
            nc.sync.dma_start(out=exmsk_t[:], in_=extra_mask[:])

            # ---------- phase 2: stream groups; exp-sum + fold ----
            bsel = sml.tile([4, 2048], F32)
            sc_g = sc[:].rearrange("r (s c) -> (r s) c", s=SUBS)  # [33*16, 8000]

            for g in range(4):
                acc4 = grp.tile([P, NCH], F32, tag="acc4")
                mapg = grp.tile([P, M], F32, tag="mapg")
                for c in range(NCH):
                    X = big.tile([P, CW], F32, tag="X")
                    nc.sync.dma_start(
                        out=X[:],
                        in_=sc_g[g * P:(g + 1) * P, c * CW:(c + 1) * CW],
                    )
                    E = big.tile([P, CW], F32, tag="E")
                    nc.scalar.activation(E[:], X[:], mybir.ActivationFunctionType.Exp,
                                         accum_out=acc4[:, c:c + 1])
                    # fold 2000 -> 125 map (window 16, contiguous)
                    x3 = X[:].rearrange("p (m w) -> p m w", w=W)
                    t1 = scr.tile([P, CW // 2], F32, tag="t1")
                    t13 = t1[:].rearrange("p (m w) -> p m w", w=8)
                    nc.vector.tensor_tensor(out=t13[:, :, :], in0=x3[:, :, 0:8],
                                            in1=x3[:, :, 8:16], op=mybir.AluOpType.max)
                    t2 = scr.tile([P, CW // 4], F32, tag="t2")
                    t23 = t2[:].rearrange("p (m w) -> p m w", w=4)
                    nc.vector.tensor_tensor(out=t23[:, :, :], in0=t13[:, :, 0:4],
                                            in1=t13[:, :, 4:8], op=mybir.AluOpType.max)
                    t3 = scr.tile([P, CW // 8], F32, tag="t3")
                    t33 = t3[:].rearrange("p (m w) -> p m w", w=2)
                    nc.vector.tensor_tensor(out=t33[:, :, :], in0=t23[:, :, 0:2],
                                            in1=t23[:, :, 2:4], op=mybir.AluOpType.max)
                    mp3 = mapg[:, c * (CW // W):(c + 1) * (CW // W)].rearrange(
                        "p (m w) -> p m w", w=1)
                    nc.vector.tensor_tensor(out=mp3[:, :, :], in0=t33[:, :, 0:1],
                                            in1=t33[:, :, 1:2], op=mybir.AluOpType.max)

                # row sums -> logZ -> offs
                s01 = grp.tile([P, 1], F32, tag="s01")
                s23 = grp.tile([P, 1], F32, tag="s23")
                stot = grp.tile([P, 1], F32, tag="stot")
                nc.vector.tensor_tensor(out=s01[:], in0=acc4[:, 0:1], in1=acc4[:, 1:2],
                                        op=mybir.AluOpType.add)
                nc.vector.tensor_tensor(out=s23[:], in0=acc4[:, 2:3], in1=acc4[:, 3:4],
                                        op=mybir.AluOpType.add)
                nc.vector.tensor_tensor(out=stot[:], in0=s01[:], in1=s23[:],
                                        op=mybir.AluOpType.add)
                ps8 = ps.tile([8, 1], F32, tag="ps8")
                nc.tensor.matmul(ps8[:], lhsT=seg16_t[:], rhs=stot[:], start=True,
                                 stop=True)
                lnz = grp.tile([8, 1], F32, tag="lnz")
                nc.scalar.activation(lnz[:], ps8[:], mybir.ActivationFunctionType.Ln)
                bg = grp.tile([8, 1], F32, tag="bg")
                nc.sync.dma_start(out=bg[:], in_=beams[8 * g:8 * g + 8, :])
                offg = grp.tile([8, 1], F32, tag="offg")
                nc.vector.tensor_tensor(out=offg[:], in0=bg[:], in1=lnz[:],
                                        op=mybir.AluOpType.subtract)
                nc.sync.dma_start(out=offscr_flat[8 * g:8 * g + 8, :], in_=offg[:])
                psb = ps.tile([P, 1], F32, tag="psb")
                nc.tensor.matmul(psb[:], lhsT=t16_t[:], rhs=offg[:], start=True,
                                 stop=True)
                offbc = grp.tile([P, 1], F32, tag="offbc")
                nc.vector.tensor_copy(out=offbc[:], in_=psb[:])

                # adjust + mask the map
                nc.vector.tensor_scalar(mapg[:], mapg[:], offbc[:, 0:1], scalar2=None,
                                        op0=mybir.AluOpType.add)
                nc.vector.tensor_tensor(out=mapg[:], in0=mapg[:],
                                        in1=mm_t[:, g * M:(g + 1) * M],
                                        op=mybir.AluOpType.add)

                # per-partition top-16 segments
                mcat = grp.tile([P, 16], F32, tag="mcat")
                icat = grp.tile([P, 16], U32, tag="icat")
                nc.vector.max(out=mcat[:, 0:8], in_=mapg[:])
                nc.vector.max_index(out=icat[:, 0:8], in_max=mcat[:, 0:8],
                                    in_values=mapg[:])
                map2 = grp.tile([P, M], F32, tag="map2")
                nc.vector.match_replace(out=map2[:], in_to_replace=mcat[:, 0:8],
                                        in_values=mapg[:], imm_value=NEG)
                nc.vector.max(out=mcat[:, 8:16], in_=map2[:])
                nc.vector.max_index(out=icat[:, 8:16], in_max=mcat[:, 8:16],
                                    in_values=map2[:])

                # flatten candidates
                nc.sync.dma_start(
                    out=bsel[g:g + 1, :],
                    in_=mcat[:],
                )
                nc.sync.dma_start(out=segscr[g:g + 1, :],
                                  in_=icat[:])

            # ---------- halo row (row 32): sumexp -> OFFSCR[32] ----
            ones_t = io.tile([P, 1], F32)
            nc.sync.dma_start(out=ones_t[:], in_=ones128[:])
            X32 = big.tile([P, V // P], F32, tag="X32")
            nc.sync.dma_start(
                out=X32[:],
                in_=sc[32:33, :].rearrange("r (p c) -> (r p) c", p=P),
            )
            E32 = big.tile([P, V // P], F32, tag="E32")
            a32 = grp.tile([P, 1], F32, tag="a32")
            nc.scalar.activation(E32[:], X32[:], mybir.ActivationFunctionType.Exp,
                                 accum_out=a32[:])
            ps1 = ps.tile([1, 1], F32, tag="ps1")
            nc.tensor.matmul(ps1[:], lhsT=ones_t[:], rhs=a32[:], start=True, stop=True)
            lnz32 = grp.tile([1, 1], F32, tag="lnz32")
            nc.scalar.activation(lnz32[:], ps1[:], mybir.ActivationFunctionType.Ln)
            bg32 = grp.tile([1, 1], F32, tag="bg32")
            nc.sync.dma_start(out=bg32[:], in_=beams[32:33, :])
            off32 = grp.tile([1, 1], F32, tag="off32")
            nc.vector.tensor_tensor(out=off32[:], in0=bg32[:], in1=lnz32[:],
                                    op=mybir.AluOpType.subtract)
            nc.sync.dma_start(out=offscr_flat[32:33, :], in_=off32[:])

            # ---------- phase 4: B-select (top-16 segments per group) ----
            bv = sml.tile([4, 16], F32)
            bp = sml.tile([4, 16], U32)
            nc.vector.max(out=bv[:, 0:8], in_=bsel[:])
            nc.vector.max_index(out=bp[:, 0:8], in_max=bv[:, 0:8], in_values=bsel[:])
            bsel2 = sml.tile([4, 2048], F32)
            nc.vector.match_replace(out=bsel2[:], in_to_replace=bv[:, 0:8],
                                    in_values=bsel[:], imm_value=NEG)
            nc.vector.max(out=bv[:, 8:16], in_=bsel2[:])
            nc.vector.max_index(out=bp[:, 8:16], in_max=bv[:, 8:16], in_values=bsel2[:])

            pos64 = sml.tile([64, 1], U32)
            nc.sync.dma_start(out=pos64[:],
                              in_=bp[:])
            pos64i = sml.tile([64, 1], I32)
            nc.vector.tensor_copy(out=pos64i[:], in_=pos64[:])

            part64 = sml.tile([64, 1], I32)
            nc.vector.tensor_scalar(part64[:], pos64i[:], 4, scalar2=None,
                                    op0=mybir.AluOpType.logical_shift_right)
            flatpos = sml.tile([64, 1], I32)
            nc.vector.tensor_tensor(out=flatpos[:], in0=pos64i[:], in1=c64_t[:, 0:1],
                                    op=mybir.AluOpType.add)
            selseg_u = sml.tile([64, 1], U32)
            nc.gpsimd.indirect_dma_start(
                out=selseg_u[:], out_offset=None, in_=segscr_flat,
                in_offset=bass.IndirectOffsetOnAxis(ap=flatpos[:, 0:1], axis=0),
            )
            selseg = sml.tile([64, 1], I32)
            nc.vector.tensor_copy(out=selseg[:], in_=selseg_u[:])

            t64 = sml.tile([64, 1], I32)
            nc.vector.tensor_scalar(t64[:], part64[:], 4, scalar2=None,
                                    op0=mybir.AluOpType.logical_shift_right)
            sub64 = sml.tile([64, 1], I32)
            nc.vector.tensor_scalar(sub64[:], part64[:], 15, scalar2=None,
                                    op0=mybir.AluOpType.bitwise_and)
            r64 = sml.tile([64, 1], I32)
            nc.vector.tensor_tensor(out=r64[:], in0=c64_t[:, 1:2], in1=t64[:],
                                    op=mybir.AluOpType.add)
            # base = r*128000 + sub*8000 + seg*16 (into scores33 flat)
            tmp1 = sml.tile([64, 1], I32)
            nc.vector.tensor_scalar(tmp1[:], r64[:], V, scalar2=None,
                                    op0=mybir.AluOpType.mult)
            tmp2 = sml.tile([64, 1], I32)
            nc.vector.tensor_scalar(tmp2[:], sub64[:], COLS, scalar2=None,
                                    op0=mybir.AluOpType.mult)
            tmp3 = sml.tile([64, 1], I32)
            nc.vector.tensor_scalar(tmp3[:], selseg[:], W, scalar2=None,
                                    op0=mybir.AluOpType.mult)
            base = sml.tile([64, 1], I32)
            nc.vector.tensor_tensor(out=base[:], in0=tmp1[:], in1=tmp2[:],
                                    op=mybir.AluOpType.add)
            nc.vector.tensor_tensor(out=base[:], in0=base[:], in1=tmp3[:],
                                    op=mybir.AluOpType.add)

            # reported word = t*V + sub*8000 + seg*16 + k - 2B
            wbase = sml.tile([64, 1], I32)
            nc.vector.tensor_scalar(wbase[:], t64[:], V, scalar2=None,
                                    op0=mybir.AluOpType.mult)
            nc.vector.tensor_tensor(out=wbase[:], in0=wbase[:], in1=tmp2[:],
                                    op=mybir.AluOpType.add)
            nc.vector.tensor_tensor(out=wbase[:], in0=wbase[:], in1=tmp3[:],
                                    op=mybir.AluOpType.add)
            nc.vector.tensor_tensor(out=wbase[:], in0=wbase[:], in1=c64_t[:, 2:3],
                                    op=mybir.AluOpType.subtract)
            wrep = sml.tile([64, W], I32)
            nc.vector.tensor_tensor(out=wrep[:], in0=k16_t[:],
                                    in1=wbase[:, 0:1].to_broadcast([64, W]),
                                    op=mybir.AluOpType.add)

            # contents gather (16 per selected segment)
            conts = sml.tile([64, W], F32)
            nc.gpsimd.indirect_dma_start(
                out=conts[:], out_offset=None, in_=sc_flat,
                in_offset=bass.IndirectOffsetOnAxis(ap=base[:, 0:1], axis=0),
            )
            offv = sml.tile([64, 1], F32)
            nc.gpsimd.indirect_dma_start(
                out=offv[:], out_offset=None, in_=offscr_flat,
                in_offset=bass.IndirectOffsetOnAxis(ap=r64[:, 0:1], axis=0),
            )
            cadj = sml.tile([64, W], F32)
            nc.vector.tensor_scalar(cadj[:], conts[:], offv[:, 0:1], scalar2=None,
                                    op0=mybir.AluOpType.add)

            # ---------- extras ----
            ev = sml.tile([P, 5], F32)
            for k in range(5):
                nc.gpsimd.indirect_dma_start(
                    out=ev[:, k:k + 1], out_offset=None, in_=sc_flat,
                    in_offset=bass.IndirectOffsetOnAxis(ap=exoff_t[:, k:k + 1], axis=0),
                )
            eoffv = sml.tile([P, 5], F32)
            for k in range(5):
                nc.gpsimd.indirect_dma_start(
                    out=eoffv[:, k:k + 1], out_offset=None, in_=offscr_flat,
                    in_offset=bass.IndirectOffsetOnAxis(ap=exrow_t[:, k:k + 1], axis=0),
                )
            evadj = sml.tile([P, 5], F32)
            nc.vector.tensor_tensor(out=evadj[:], in0=ev[:], in1=eoffv[:],
                                    op=mybir.AluOpType.add)
            nc.vector.tensor_tensor(out=evadj[:], in0=evadj[:], in1=exmsk_t[:],
                                    op=mybir.AluOpType.add)

            # ---------- final merge ----
            fin = sml.tile([4, NFIN], F32)
            nc.sync.dma_start(
                out=fin[:, 0:256],
                in_=cadj[:],
            )
            nc.sync.dma_start(
                out=fin[:, 256:NFIN],
                in_=evadj[:],
            )
            nc.sync.dma_start(out=wrdscr[:, 0:256],
                              in_=wrep[:])
            nc.sync.dma_start(out=wrdscr[:, 256:NFIN],
                              in_=exwrd_t[:])

            fs = sml.tile([4, 16], F32)
            fq = sml.tile([4, 16], U32)
            nc.vector.max(out=fs[:, 0:8], in_=fin[:])
            nc.vector.max_index(out=fq[:, 0:8], in_max=fs[:, 0:8], in_values=fin[:])
            fin2 = sml.tile([4, NFIN], F32)
            nc.vector.match_replace(out=fin2[:], in_to_replace=fs[:, 0:8],
                                    in_values=fin[:], imm_value=NEG)
            nc.vector.max(out=fs[:, 8:16], in_=fin2[:])
            nc.vector.max_index(out=fq[:, 8:16], in_max=fs[:, 8:16], in_values=fin2[:])

            nc.sync.dma_start(out=out_s[:], in_=fs[:])

            q64 = sml.tile([64, 1], U32)
            nc.sync.dma_start(out=q64[:],
                              in_=fq[:])
            q64i = sml.tile([64, 1], I32)
            nc.vector.tensor_copy(out=q64i[:], in_=q64[:])
            flatq = sml.tile([64, 1], I32)
            nc.vector.tensor_tensor(out=flatq[:], in0=q64i[:], in1=c64_t[:, 3:4],
                                    op=mybir.AluOpType.add)
            wsel = sml.tile([64, 1], I32)
            nc.gpsimd.indirect_dma_start(
                out=wsel[:], out_offset=None, in_=wrdscr_flat,
                in_offset=bass.IndirectOffsetOnAxis(ap=flatq[:, 0:1], axis=0),
            )
            nc.sync.dma_start(out=out_w[:],
                              in_=wsel[:])

    nc.compile()
    return nc


def _get_program():
    if "nc" not in _prog_cache:
        _prog_cache["nc"] = _build_program()
    return _prog_cache["nc"]


# --------------------------------------------------------------------------
# public entry
# --------------------------------------------------------------------------
def kernel(scores, beam_scores, input_ids, num_beams):
    scores = np.ascontiguousarray(np.asarray(scores, dtype=np.float32))
    beam_scores = np.asarray(beam_scores, dtype=np.float32)
    input_ids = np.asarray(input_ids)
    nb = int(num_beams)
    assert scores.shape == (256, V) and nb == TOK

    nc = _get_program()

    in_maps = []
    for c in range(8):
        rows = slice(32 * c, 32 * c + 32)
        s33 = np.zeros((NROW, V), np.float32)
        s33[:32] = scores[rows]
        i33 = np.zeros((NROW, 128), np.int64)
        i33[:32] = input_ids[rows]
        b33 = np.zeros((NROW, 1), np.float32)
        b33[:32, 0] = beam_scores[rows]
        if c < 7:
            s33[32] = scores[32 * c + 32]
            i33[32] = input_ids[32 * c + 32]
            b33[32, 0] = beam_scores[32 * c + 32]
        aux = _host_prep(c, i33)
        in_maps.append({
            "scores33": s33,
            "ids_off": aux["ids_off"],
            "beams33": b33,
            "mapmask": aux["mapmask"],
            "extra_off": aux["extra_off"],
            "extra_row": aux["extra_row"],
            "extra_words": aux["extra_words"],
            "extra_mask": aux["extra_mask"],
            "seg16": aux["seg16"],
            "t16": aux["t16"],
            "c64": aux["c64"],
            "k16": aux["k16"],
            "ones128": np.ones((P, 1), np.float32),
        })

    global _last_in_maps
    _last_in_maps = in_maps
    res = run_bass_kernel_spmd(nc, in_maps, list(range(8)))
    next_scores = np.concatenate([r["out_scores"] for r in res.results], axis=0)
    next_words = np.concatenate([r["out_words"] for r in res.results], axis=0)
    return next_scores.astype(np.float32), next_words.astype(np.int32)
